# revision 40
# baseline (speedup 1.0000x reference)
"""AAConv (attention-augmented conv) Trainium2 kernel, 8-core data-parallel.

Reference shapes: x (16,256,32,32) f32
  conv branch: 3x3 SAME conv 256->128 (+bias)
  attn branch: 1x1 qkv conv (k|q|v = 128|128|128 rows of qkv_w), 8 heads d=16,
               softmax attention over 1024 positions, 1x1 proj 128->128 (+bias)
  out = concat([conv_out, attn_out], axis=1) -> (16,256,32,32)

Sharding: pure data-parallel over batch. Each of 8 cores gets 2 images and
all weights; outputs concatenated on host.

Per-core design (channels on partitions, pixels on free dim):
 - logits computed transposed, L^T[k,q] (lhsT=K_h [16,128], rhs=Q_h [16,512]),
   4-way row-tiled over heads (K=16 contraction, tile_position=(32h,0)).
   K/Q stored padded: head h at partitions 32h..32h+16 (zero pad rows) so
   lhsT/rhs share base partitions; produced by M=128 matmuls against
   zero-padded transposed weights.
 - softmax denominator via a ones-column appended to V^T in the AV matmul
   (M=17 col-tiled, tile_position=(0,32h)): no cross-partition reductions.
 - exp on ScalarE (the bottleneck engine, ~128us/core) straight out of PSUM.
 - softmax normalize: strided-partition reciprocal on DVE + gpsimd
   partition_broadcast + one strided multiply; attn kept in the padded
   partition layout and proj done with zero-padded proj weights.
 - conv as 9 shift-matmuls over a zero-padded 34x34 spatial buffer.
 - all matmuls bf16 (f32 psum accumulate).
"""

import sys

for p in ("/opt/trn_rl_repo",):
    if p not in sys.path:
        sys.path.insert(0, p)

import numpy as np

import concourse.bass as bass
import concourse.tile as tile
from concourse import bacc, mybir
from concourse.masks import make_identity

F32 = mybir.dt.float32
BF16 = mybir.dt.bfloat16
AF = mybir.ActivationFunctionType

# Problem dims (hardcoded)
B, C, H, W = 16, 256, 32, 32
HW = H * W                      # 1024
CO, DK, DV, NH = 256, 128, 128, 8
D = DK // NH                    # 16 head dim
CONV_CO = CO - DV               # 128
N_CORES = 8
BL = B // N_CORES               # 2 images per core
HP = H + 2                      # 34 padded
PADHW = HP * HP                 # 1156
SCALE = float(D) ** -0.5        # 0.25


def hview(ap):
    """[128, N] -> [4, 32, N] partition-group view."""
    return ap.rearrange("(a b) n -> a b n", b=32)


def build_nc():
    nc = bacc.Bacc("TRN2", target_bir_lowering=False, debug=False,
                   num_devices=N_CORES)

    x_ext = nc.declare_dram_parameter("x", [BL, C, HW], F32, isOutput=False)
    convw_ext = nc.declare_dram_parameter("conv_w", [9, C, CONV_CO], F32, isOutput=False)
    convb_ext = nc.declare_dram_parameter("conv_b", [1, CONV_CO], F32, isOutput=False)
    qkvw_ext = nc.declare_dram_parameter("qkv_w", [2 * DK + DV, C], F32, isOutput=False)
    qkvb_ext = nc.declare_dram_parameter("qkv_b", [1, 2 * DK + DV], F32, isOutput=False)
    projw_ext = nc.declare_dram_parameter("proj_w", [DV, DV], F32, isOutput=False)
    projb_ext = nc.declare_dram_parameter("proj_b", [1, DV], F32, isOutput=False)
    out_ext = nc.declare_dram_parameter("out", [BL, CO, HW], F32, isOutput=True)

    with tile.TileContext(nc) as tc:
        with (
            tc.tile_pool(name="const", bufs=1) as constp,
            tc.tile_pool(name="stage", bufs=1) as stagep,
            tc.tile_pool(name="img", bufs=2) as imgp,
            tc.tile_pool(name="st", bufs=3) as stp,
            tc.tile_pool(name="psum", bufs=1, space="PSUM") as psp,
        ):
            # ---------------- weights ----------------
            ident = constp.tile([128, 128], F32)
            make_identity(nc, ident[:])

            # conv weights: natural [c, o] per tap, bf16. cols (ct*9+t)*128+o
            wconv_f32 = stagep.tile([128, 2 * 9 * CONV_CO], F32)
            wconv = constp.tile([128, 2 * 9 * CONV_CO], BF16)
            for ct in range(2):
                for t in range(9):
                    blk = slice((ct * 9 + t) * 128, (ct * 9 + t + 1) * 128)
                    nc.sync.dma_start(wconv_f32[:, blk],
                                      convw_ext[t, ct * 128:(ct + 1) * 128, :])
                    nc.vector.tensor_copy(wconv[:, blk], wconv_f32[:, blk])

            # qkv weights: DMA natural [chan, c], PE-transpose to [c, chan],
            # scatter into zero-padded layouts.
            qkvw_sb = stagep.tile([128, 3 * C], F32)  # blk b at cols b*256
            for blk in range(3):
                nc.sync.dma_start(
                    qkvw_sb[:, blk * C:(blk + 1) * C],
                    qkvw_ext[blk * 128:(blk + 1) * 128, :],
                )
            # wkq_pad: [128 c, ct*512 + tgt*256 + hh*128 + hp*32 + d] bf16, zero pad
            wkq_pad = constp.tile([128, 2 * 512], BF16)
            nc.gpsimd.memset(wkq_pad[:], 0.0)
            wvT = constp.tile([128, 2 * 128], BF16)
            for ct in range(2):
                for blk in range(3):
                    tps = psp.tile([128, 128], F32, tag="ms", name=f"tps_{ct}_{blk}")
                    nc.tensor.transpose(
                        tps[:], qkvw_sb[:, blk * C + ct * 128: blk * C + (ct + 1) * 128],
                        ident[:])
                    if blk < 2:
                        dst = wkq_pad[:, ct * 512 + blk * 256:
                                      ct * 512 + (blk + 1) * 256].rearrange(
                            "p (h d) -> p h d", d=32)[:, :, 0:16]
                        src = tps[:].rearrange("p (h d) -> p h d", d=16)
                        nc.vector.tensor_copy(dst, src)
                    else:
                        nc.vector.tensor_copy(
                            wvT[:, ct * 128:(ct + 1) * 128], tps[:])

            projw_sb = stagep.tile([128, 128], F32)
            nc.sync.dma_start(projw_sb[:], projw_ext[:])
            # padded projT: rows 32hp+16+d = proj_w^T row (4hh+hp)*16+d, rest 0
            # (matches the attn_pad layout where attn lives at rows 32hp+16..32).
            # Column-scatter proj_w in free space first, then PE-transpose.
            projw_pad = stagep.tile([128, 2 * 128], F32)
            nc.gpsimd.memset(projw_pad[:], 0.0)
            for hh in range(2):
                nc.vector.tensor_copy(
                    projw_pad[:, hh * 128:(hh + 1) * 128].rearrange(
                        "p (a b) -> p a b", b=32)[:, :, 16:32],
                    projw_sb[:, 64 * hh:64 * (hh + 1)].rearrange(
                        "p (a b) -> p a b", b=16))
            projT_pad = constp.tile([128, 2 * 128], BF16)
            for hh in range(2):
                tps2 = psp.tile([128, 128], F32, tag="ms", name=f"tps2_{hh}")
                nc.tensor.transpose(
                    tps2[:], projw_pad[:, hh * 128:(hh + 1) * 128], ident[:])
                nc.vector.tensor_copy(projT_pad[:, hh * 128:(hh + 1) * 128],
                                      tps2[:])

            # k/q biases as a padded bias ROW (free-dim scatter, all DVE-legal),
            # added to kqps via a K=1 matmul against a ones row.
            qkvb_sb = stagep.tile([1, 2 * DK + DV], F32)
            nc.sync.dma_start(qkvb_sb[:], qkvb_ext[:])
            brow_pad = constp.tile([1, 512], BF16)
            nc.gpsimd.memset(brow_pad[:], 0.0)
            for tgt in range(2):
                for hh in range(2):
                    nc.vector.tensor_copy(
                        brow_pad[0:1, (tgt * 2 + hh) * 128:
                                 (tgt * 2 + hh + 1) * 128].rearrange(
                            "p (a b) -> p a b", b=32)[:, :, 0:16],
                        qkvb_sb[0:1, tgt * DK + 64 * hh: tgt * DK + 64 * (hh + 1)
                                ].rearrange("p (a b) -> p a b", b=16))
            ones512 = constp.tile([1, 512], BF16)
            nc.gpsimd.memset(ones512[:], 1.0)
            convb_f32 = stagep.tile([1, CONV_CO], F32)
            nc.sync.dma_start(convb_f32[:], convb_ext[:])
            convb_row = constp.tile([1, CONV_CO], BF16)
            nc.vector.tensor_copy(convb_row[:], convb_f32[:])
            projb_f32 = stagep.tile([1, DV], F32)
            nc.sync.dma_start(projb_f32[:], projb_ext[:])
            projb_row = constp.tile([1, DV], BF16)
            nc.vector.tensor_copy(projb_row[:], projb_f32[:])

            # v-bias broadcast to 128 partitions via PE (ones ⊗ bv)
            bv_f32 = stagep.tile([1, DV], F32)
            nc.sync.dma_start(bv_f32[:], qkvb_ext[:, 2 * DK:])
            bv_bf = stagep.tile([1, DV], BF16)
            nc.vector.tensor_copy(bv_bf[:], bv_f32[:])
            ones_row = constp.tile([1, 128], BF16)
            nc.gpsimd.memset(ones_row[:], 1.0)
            ones_q = constp.tile([128, 32], F32)
            nc.gpsimd.memset(ones_q[:], 1.0)
            bvps = psp.tile([128, 128], F32, tag="ms")
            nc.tensor.matmul(bvps[:], ones_row[:], bv_bf[:], start=True, stop=True)
            bv_bc = constp.tile([128, 128], F32)
            nc.vector.tensor_copy(bv_bc[:], bvps[:])

            # ---------------- per image ----------------
            PADW = PADHW + 36   # room for the last conv chunk's shifted reads
            import os as _os
            _reps = int(_os.environ.get("AACONV_BENCH_REPS", "1"))
            for img in range(BL * _reps):
                imgd = img % BL
                xin = imgp.tile([128, 2 * HW], F32, tag="xin", name=f"xin_{img}")
                for ct in range(2):
                    nc.sync.dma_start(
                        xin[:, ct * HW:(ct + 1) * HW],
                        x_ext[imgd, ct * 128:(ct + 1) * 128, :])
                # compact bf16 copy (KQ rhs / VT lhsT need single-run APs)
                x_bf = imgp.tile([128, 2 * HW], BF16, tag="xbf", name=f"xbf_{img}")
                nc.vector.tensor_copy(x_bf[:], xin[:])
                # zero-padded 34x34 layout for the conv, filled via DMA
                xpad = imgp.tile([128, 2 * PADW], BF16, tag="xpad", name=f"xpad_{img}")
                nc.gpsimd.memset(xpad[:], 0.0)
                for ct in range(2):
                    nc.sync.dma_start(
                        xpad[:, ct * PADW: ct * PADW + PADHW].rearrange(
                            "p (h w) -> p h w", h=HP)[:, 1:33, 1:33],
                        x_bf[:, ct * HW:(ct + 1) * HW].rearrange(
                            "p (h w) -> p h w", h=H))

                # ---- K_pad / Q_pad ----
                k_pad = imgp.tile([128, 2 * HW], BF16, tag="kpad", name=f"kpad_{img}")
                q_pad = imgp.tile([128, 2 * HW], BF16, tag="qpad", name=f"qpad_{img}")
                for tgt, dst in ((0, k_pad), (1, q_pad)):
                    for hh in range(2):
                        kqps = psp.tile([128, HW], F32, tag="ms",
                                        name=f"kqps_{img}_{tgt}_{hh}")
                        for qn in range(2):
                            for ct in range(2):
                                nc.tensor.matmul(
                                    kqps[:, qn * 512:(qn + 1) * 512],
                                    wkq_pad[:, ct * 512 + tgt * 256 + hh * 128:
                                            ct * 512 + tgt * 256 + (hh + 1) * 128],
                                    x_bf[:, ct * HW + qn * 512:
                                         ct * HW + (qn + 1) * 512],
                                    start=(ct == 0), stop=False)
                            nc.tensor.matmul(
                                kqps[:, qn * 512:(qn + 1) * 512],
                                brow_pad[0:1, (tgt * 2 + hh) * 128:
                                         (tgt * 2 + hh + 1) * 128],
                                ones512[0:1, :],
                                start=False, stop=True)
                        nc.vector.tensor_copy(
                            dst[:, hh * HW:(hh + 1) * HW], kqps[:])

                # ---- V^T with ones column, 32-stride padded blocks ----
                # vt_aug block (hh,kt) at cols (hh*8+kt)*128 + hp*32 +
                #   [0 = ones, 1:16 = zeros, 16:32 = V_h]  (M=32 AV matmuls
                #   write full PSUM quadrants; denominator lands on quadrant
                #   rows 32hp, attn on rows 32hp+16..32)
                vt_aug = imgp.tile([128, 2 * 8 * 128], BF16, tag="vtaug",
                                   name=f"vtaug_{img}")
                nc.gpsimd.memset(vt_aug[:], 0.0)
                nc.gpsimd.memset(
                    vt_aug[:].rearrange("p (g d) -> p g d", d=32)[:, :, 0:1], 1.0)
                for kt in range(8):
                    vtps = psp.tile([128, 128], F32, tag="ms", name=f"vtps_{img}_{kt}")
                    for ct in range(2):
                        nc.tensor.matmul(
                            vtps[:],
                            x_bf[:, ct * HW + kt * 128: ct * HW + (kt + 1) * 128],
                            wvT[:, ct * 128:(ct + 1) * 128],
                            start=(ct == 0), stop=(ct == 1))
                    for hh in range(2):
                        base = (hh * 8 + kt) * 128
                        dst = vt_aug[:, base: base + 128].rearrange(
                            "p (h d) -> p h d", d=32)[:, :, 16:32]
                        src = vtps[:, hh * 64:(hh + 1) * 64].rearrange(
                            "p (h d) -> p h d", d=16)
                        bvb = bv_bc[:, hh * 64:(hh + 1) * 64].rearrange(
                            "p (h d) -> p h d", d=16)
                        nc.vector.tensor_add(dst, src, bvb)

                # ---- conv branch ----
                # computed over the padded flat space in row-aligned chunks so
                # every matmul rhs is a single contiguous run; junk columns
                # (x=32,33 of each padded row) are skipped on evacuation.
                out_conv = imgp.tile([128, HW], F32, tag="oconv", name=f"oconv_{img}")
                for (r0, nr) in ((0, 15), (15, 15), (30, 2)):
                    n = (nr - 1) * HP + W          # chunk free size (<=512)
                    cs = (r0 + 1) * HP + 1         # pad-flat offset of (r0, 0)
                    cvps = psp.tile([128, 512], F32, tag="ms",
                                    name=f"cvps_{img}_{r0}")
                    for t in range(9):
                        dy, dx = t // 3, t % 3
                        sh = (dy - 1) * HP + (dx - 1)
                        for ct in range(2):
                            nc.tensor.matmul(
                                cvps[:, 0:n],
                                wconv[:, (ct * 9 + t) * 128:(ct * 9 + t + 1) * 128],
                                xpad[:, ct * PADW + cs + sh: ct * PADW + cs + sh + n],
                                start=((t, ct) == (0, 0)), stop=False)
                    nc.tensor.matmul(cvps[:, 0:n], convb_row[:], ones512[0:1, 0:n],
                                     start=False, stop=True)
                    nc.vector.tensor_copy(
                        out_conv[:, r0 * W:(r0 + nr) * W].rearrange(
                            "p (h w) -> p h w", h=nr),
                        cvps[:, 0:nr * HP].rearrange(
                            "p (h w) -> p h w", w=HP)[:, :, 0:W])
                nc.sync.dma_start(out_ext[imgd, 0:CONV_CO, :], out_conv[:])

                # ---- attention ----
                # attn_pad: [128, 2*HW] bf16, hh at cols hh*HW; head hp's
                # normalized attn at rows 32hp+16..32 (fully written by the
                # normalize muls, so no memset needed)
                attn_pad = imgp.tile([128, 2 * HW], BF16, tag="attnp",
                                     name=f"attnp_{img}")
                rrec = imgp.tile([128, HW], F32, tag="rrec", name=f"rrec_{img}")
                rd16 = imgp.tile([128, 2 * HW], F32, tag="rd16", name=f"rd16_{img}")
                for hh in range(2):
                    av = psp.tile([128, HW], F32, tag="av", name=f"av_{img}_{hh}")
                    for kt in range(8):
                        for qh in range(2):
                            lgs = []
                            for hg in range(2):
                                lg = psp.tile([128, 1024], F32, tag="lg", bufs=2,
                                              name=f"lg_{img}_{hh}_{kt}_{qh}_{hg}")
                                lgs.append(lg)
                                for j in range(2):
                                    hp = 2 * hg + j
                                    nc.tensor.matmul(
                                        lg[:, j * 512:(j + 1) * 512],
                                        k_pad[32 * hp:32 * hp + 16,
                                              hh * HW + kt * 128: hh * HW + (kt + 1) * 128],
                                        q_pad[32 * hp:32 * hp + 16,
                                              hh * HW + qh * 512: hh * HW + (qh + 1) * 512],
                                        start=True, stop=True,
                                        tile_position=(32 * hp, 0))
                            sts = []
                            for hg in range(2):
                                st = stp.tile([128, 1024], BF16, tag="st",
                                              name=f"st_{img}_{hh}_{kt}_{qh}_{hg}")
                                sts.append(st)
                                nc.scalar.activation(st[:], lgs[hg][:], AF.Exp,
                                                     scale=SCALE)
                            for hg in range(2):
                                for j in range(2):
                                    hp = 2 * hg + j
                                    base = (hh * 8 + kt) * 128 + 32 * hp
                                    nc.tensor.matmul(
                                        av[32 * hp:32 * hp + 32,
                                           qh * 512:(qh + 1) * 512],
                                        vt_aug[:, base: base + 32],
                                        sts[hg][:, j * 512:(j + 1) * 512],
                                        start=(kt == 0), stop=(kt == 7),
                                        skip_group_check=True,
                                        tile_position=(0, 32 * hp))
                    # normalize + evacuate into padded attn layout.
                    # av rows per quadrant hp: 32hp = den, 32hp+1..16 = 0,
                    # 32hp+16..32 = unnormalized attn. Full-tile ops only.
                    for qh in range(2):
                        sl = slice(qh * 512, (qh + 1) * 512)
                        slh = slice(hh * HW + qh * 512, hh * HW + (qh + 1) * 512)
                        nc.vector.reciprocal(rrec[:, sl], av[:, sl])
                        # broadcast each quadrant's 1/den row via K=1 matmul
                        rdps = psp.tile([128, 512], F32, tag="ms",
                                        name=f"rdps_{img}_{hh}_{qh}")
                        for hp in range(4):
                            nc.tensor.matmul(
                                rdps[32 * hp:32 * hp + 32, :],
                                ones_q[32 * hp:32 * hp + 1, :],
                                rrec[32 * hp:32 * hp + 1, sl],
                                start=True, stop=True,
                                tile_position=(32 * hp, 32 * hp))
                        nc.vector.tensor_copy(rd16[:, slh], rdps[:])
                        nc.vector.tensor_mul(attn_pad[:, slh], av[:, sl],
                                             rd16[:, slh])

                # ---- proj (padded weights over both halves) ----
                projps = psp.tile([128, HW], F32, tag="ms", name=f"projps_{img}")
                for qn in range(2):
                    for hh in range(2):
                        nc.tensor.matmul(
                            projps[:, qn * 512:(qn + 1) * 512],
                            projT_pad[:, hh * 128:(hh + 1) * 128],
                            attn_pad[:, hh * HW + qn * 512: hh * HW + (qn + 1) * 512],
                            start=(hh == 0), stop=False)
                    nc.tensor.matmul(projps[:, qn * 512:(qn + 1) * 512],
                                     projb_row[:], ones512[0:1, :],
                                     start=False, stop=True)
                out_proj = imgp.tile([128, HW], F32, tag="oproj", name=f"oproj_{img}")
                nc.vector.tensor_copy(out_proj[:], projps[:])
                nc.sync.dma_start(out_ext[imgd, CONV_CO:, :], out_proj[:])

    return nc


_NC = None


def _get_nc():
    global _NC
    if _NC is None:
        _NC = build_nc()
        _NC.compile()
    return _NC


def kernel(**inputs):
    from concourse.bass_utils import run_bass_kernel_spmd

    nc = _get_nc()
    x = np.asarray(inputs["x"], np.float32).reshape(B, C, HW)
    conv_w = np.ascontiguousarray(np.asarray(inputs["conv_w"], np.float32).reshape(9, C, CONV_CO))
    conv_b = np.ascontiguousarray(np.asarray(inputs["conv_b"], np.float32).reshape(1, CONV_CO))
    qkv_w = np.ascontiguousarray(np.asarray(inputs["qkv_w"], np.float32))
    qkv_b = np.ascontiguousarray(np.asarray(inputs["qkv_b"], np.float32).reshape(1, 2 * DK + DV))
    proj_w = np.ascontiguousarray(np.asarray(inputs["proj_w"], np.float32))
    proj_b = np.ascontiguousarray(np.asarray(inputs["proj_b"], np.float32).reshape(1, DV))

    in_maps = []
    for i in range(N_CORES):
        in_maps.append({
            "x": np.ascontiguousarray(x[i * BL:(i + 1) * BL]),
            "conv_w": conv_w, "conv_b": conv_b,
            "qkv_w": qkv_w, "qkv_b": qkv_b,
            "proj_w": proj_w, "proj_b": proj_b,
        })
    res = run_bass_kernel_spmd(nc, in_maps, core_ids=list(range(N_CORES)))
    outs = [np.asarray(res.results[i]["out"]).reshape(BL, CO, H, W)
            for i in range(N_CORES)]
    return np.concatenate(outs, axis=0).astype(np.float32)


if __name__ == "__main__":
    nc = build_nc()
    nc.compile()
    print("built ok; instructions:", len(nc.inst_map))


# revision 53
# speedup vs baseline: 61.1270x; 61.1270x over previous
"""AAConv (attention-augmented conv) Trainium2 kernel, 8-core data-parallel.

Reference shapes: x (16,256,32,32) f32
  conv branch: 3x3 SAME conv 256->128 (+bias)
  attn branch: 1x1 qkv conv (k|q|v = 128|128|128 rows of qkv_w), 8 heads d=16,
               softmax attention over 1024 positions, 1x1 proj 128->128 (+bias)
  out = concat([conv_out, attn_out], axis=1) -> (16,256,32,32)

Sharding: pure data-parallel over batch. Each of 8 cores gets 2 images and
all weights; outputs concatenated on host.

Per-core design (channels on partitions, pixels on free dim):
 - logits computed transposed, L^T[k,q] (lhsT=K_h [16,128], rhs=Q_h [16,512]),
   4-way row-tiled over heads (K=16 contraction, tile_position=(32h,0)).
   K/Q stored padded: head h at partitions 32h..32h+16 (zero pad rows) so
   lhsT/rhs share base partitions; produced by M=128 matmuls against
   zero-padded transposed weights.
 - softmax denominator via a ones-column appended to V^T in the AV matmul
   (M=17 col-tiled, tile_position=(0,32h)): no cross-partition reductions.
 - exp on ScalarE (the bottleneck engine, ~128us/core) straight out of PSUM.
 - softmax normalize: strided-partition reciprocal on DVE + gpsimd
   partition_broadcast + one strided multiply; attn kept in the padded
   partition layout and proj done with zero-padded proj weights.
 - conv as 9 shift-matmuls over a zero-padded 34x34 spatial buffer.
 - all matmuls bf16 (f32 psum accumulate).
"""

import sys

for p in ("/opt/trn_rl_repo",):
    if p not in sys.path:
        sys.path.insert(0, p)

import numpy as np

import concourse.bass as bass
import concourse.tile as tile
from concourse import bacc, mybir
from concourse.masks import make_identity

F32 = mybir.dt.float32
BF16 = mybir.dt.bfloat16
AF = mybir.ActivationFunctionType

# Problem dims (hardcoded)
B, C, H, W = 16, 256, 32, 32
HW = H * W                      # 1024
CO, DK, DV, NH = 256, 128, 128, 8
D = DK // NH                    # 16 head dim
CONV_CO = CO - DV               # 128
N_CORES = 8
BL = B // N_CORES               # 2 images per core
HP = H + 2                      # 34 padded
PADHW = HP * HP                 # 1156
SCALE = float(D) ** -0.5        # 0.25


def hview(ap):
    """[128, N] -> [4, 32, N] partition-group view."""
    return ap.rearrange("(a b) n -> a b n", b=32)


def build_nc():
    nc = bacc.Bacc("TRN2", target_bir_lowering=False, debug=False,
                   num_devices=N_CORES)

    x_ext = nc.declare_dram_parameter("x", [BL, C, HW], F32, isOutput=False)
    convw_ext = nc.declare_dram_parameter("conv_w", [9, C, CONV_CO], F32, isOutput=False)
    convb_ext = nc.declare_dram_parameter("conv_b", [1, CONV_CO], F32, isOutput=False)
    qkvw_ext = nc.declare_dram_parameter("qkv_w", [2 * DK + DV, C], F32, isOutput=False)
    qkvb_ext = nc.declare_dram_parameter("qkv_b", [1, 2 * DK + DV], F32, isOutput=False)
    projw_ext = nc.declare_dram_parameter("proj_w", [DV, DV], F32, isOutput=False)
    projb_ext = nc.declare_dram_parameter("proj_b", [1, DV], F32, isOutput=False)
    out_ext = nc.declare_dram_parameter("out", [BL, CO, HW], F32, isOutput=True)

    with tile.TileContext(nc) as tc:
        with (
            tc.tile_pool(name="const", bufs=1) as constp,
            tc.tile_pool(name="stage", bufs=1) as stagep,
            tc.tile_pool(name="img", bufs=3) as imgp,
            tc.tile_pool(name="st", bufs=6) as stp,
            tc.tile_pool(name="psum", bufs=2, space="PSUM") as psp,
        ):
            # ---------------- weights ----------------
            ident = constp.tile([128, 128], F32)
            make_identity(nc, ident[:])

            # qkv weights: DMA natural [chan, c], PE-transpose to [c, chan],
            # scatter into zero-padded layouts.
            qkvw_sb = stagep.tile([128, 3 * C], F32)  # blk b at cols b*256
            for blk in range(3):
                nc.sync.dma_start(
                    qkvw_sb[:, blk * C:(blk + 1) * C],
                    qkvw_ext[blk * 128:(blk + 1) * 128, :],
                )

            # prefetch all images' x early (longest DMA pole) + compact bf16
            import os as _os
            _reps = int(_os.environ.get("AACONV_BENCH_REPS", "1"))
            xs_bf = []
            for img in range(BL * _reps):
                xin = imgp.tile([128, 2 * HW], F32, tag="xin", name=f"xin_{img}")
                for ct in range(2):
                    nc.sync.dma_start(
                        xin[:, ct * HW:(ct + 1) * HW],
                        x_ext[img % BL, ct * 128:(ct + 1) * 128, :])
                x_bf = imgp.tile([128, 2 * HW], BF16, tag="xbf", name=f"xbf_{img}")
                nc.vector.tensor_copy(x_bf[:], xin[:])
                xs_bf.append(x_bf)

            # wkq_pad: [128 c, ct*512 + tgt*256 + hh*128 + hp*32 + d] bf16, zero pad
            wkq_pad = constp.tile([128, 2 * 512], BF16)
            nc.gpsimd.memset(wkq_pad[:], 0.0)
            wvT = constp.tile([128, 2 * 128], BF16)
            for ct in range(2):
                for blk in range(3):
                    tps = psp.tile([128, 128], F32, tag="ms", name=f"tps_{ct}_{blk}")
                    nc.tensor.transpose(
                        tps[:], qkvw_sb[:, blk * C + ct * 128: blk * C + (ct + 1) * 128],
                        ident[:])
                    if blk < 2:
                        dst = wkq_pad[:, ct * 512 + blk * 256:
                                      ct * 512 + (blk + 1) * 256].rearrange(
                            "p (h d) -> p h d", d=32)[:, :, 0:16]
                        src = tps[:].rearrange("p (h d) -> p h d", d=16)
                        nc.vector.tensor_copy(dst, src)
                    else:
                        nc.vector.tensor_copy(
                            wvT[:, ct * 128:(ct + 1) * 128], tps[:])

            projw_sb = stagep.tile([128, 128], F32)
            nc.sync.dma_start(projw_sb[:], projw_ext[:])
            # padded projT: rows 32hp+16+d = proj_w^T row (4hh+hp)*16+d, rest 0
            # (matches the attn_pad layout where attn lives at rows 32hp+16..32).
            # Column-scatter proj_w in free space first, then PE-transpose.
            projw_pad = stagep.tile([128, 2 * 128], F32)
            nc.gpsimd.memset(projw_pad[:], 0.0)
            for hh in range(2):
                nc.vector.tensor_copy(
                    projw_pad[:, hh * 128:(hh + 1) * 128].rearrange(
                        "p (a b) -> p a b", b=32)[:, :, 16:32],
                    projw_sb[:, 64 * hh:64 * (hh + 1)].rearrange(
                        "p (a b) -> p a b", b=16))
            projT_pad = constp.tile([128, 2 * 128], BF16)
            for hh in range(2):
                tps2 = psp.tile([128, 128], F32, tag="ms", name=f"tps2_{hh}")
                nc.tensor.transpose(
                    tps2[:], projw_pad[:, hh * 128:(hh + 1) * 128], ident[:])
                nc.vector.tensor_copy(projT_pad[:, hh * 128:(hh + 1) * 128],
                                      tps2[:])

            # k/q biases as a padded bias ROW (free-dim scatter, all DVE-legal),
            # added to kqps via a K=1 matmul against a ones row.
            qkvb_sb = stagep.tile([1, 2 * DK + DV], F32)
            nc.sync.dma_start(qkvb_sb[:], qkvb_ext[:])
            brow_pad = constp.tile([1, 512], BF16)
            nc.gpsimd.memset(brow_pad[:], 0.0)
            for tgt in range(2):
                for hh in range(2):
                    nc.vector.tensor_copy(
                        brow_pad[0:1, (tgt * 2 + hh) * 128:
                                 (tgt * 2 + hh + 1) * 128].rearrange(
                            "p (a b) -> p a b", b=32)[:, :, 0:16],
                        qkvb_sb[0:1, tgt * DK + 64 * hh: tgt * DK + 64 * (hh + 1)
                                ].rearrange("p (a b) -> p a b", b=16))
            ones512 = constp.tile([1, 512], BF16)
            nc.gpsimd.memset(ones512[:], 1.0)
            convb_f32 = stagep.tile([1, CONV_CO], F32)
            nc.sync.dma_start(convb_f32[:], convb_ext[:])
            convb_row = constp.tile([1, CONV_CO], BF16)
            nc.vector.tensor_copy(convb_row[:], convb_f32[:])
            projb_f32 = stagep.tile([1, DV], F32)
            nc.sync.dma_start(projb_f32[:], projb_ext[:])
            projb_row = constp.tile([1, DV], BF16)
            nc.vector.tensor_copy(projb_row[:], projb_f32[:])

            # v-bias broadcast to 128 partitions via PE (ones ⊗ bv)
            bv_f32 = stagep.tile([1, DV], F32)
            nc.sync.dma_start(bv_f32[:], qkvb_ext[:, 2 * DK:])
            bv_bf = stagep.tile([1, DV], BF16)
            nc.vector.tensor_copy(bv_bf[:], bv_f32[:])
            ones_row = constp.tile([1, 128], BF16)
            nc.gpsimd.memset(ones_row[:], 1.0)
            ones_q = constp.tile([128, 32], BF16)
            nc.gpsimd.memset(ones_q[:], 1.0)
            bvps = psp.tile([128, 128], F32, tag="ms")
            nc.tensor.matmul(bvps[:], ones_row[:], bv_bf[:], start=True, stop=True)
            bv_bc = constp.tile([128, 128], F32)
            nc.vector.tensor_copy(bv_bc[:], bvps[:])

            # conv weights: natural [c, o] per tap, bf16. cols (ct*9+t)*128+o
            wconv_f32 = stagep.tile([128, 2 * 9 * CONV_CO], F32)
            wconv = constp.tile([128, 2 * 9 * CONV_CO], BF16)
            for ct in range(2):
                for t in range(9):
                    blk = slice((ct * 9 + t) * 128, (ct * 9 + t + 1) * 128)
                    nc.sync.dma_start(wconv_f32[:, blk],
                                      convw_ext[t, ct * 128:(ct + 1) * 128, :])
                    nc.vector.tensor_copy(wconv[:, blk], wconv_f32[:, blk])

            # ---------------- per image ----------------
            PADW = PADHW + 36   # room for the last conv chunk's shifted reads
            for img in range(BL * _reps):
                imgd = img % BL
                x_bf = xs_bf[img]
                # zero-padded 34x34 layout for the conv, filled via DMA
                xpad = imgp.tile([128, 2 * PADW], BF16, tag="xpad", name=f"xpad_{img}")
                nc.gpsimd.memset(xpad[:], 0.0)
                for ct in range(2):
                    nc.sync.dma_start(
                        xpad[:, ct * PADW: ct * PADW + PADHW].rearrange(
                            "p (h w) -> p h w", h=HP)[:, 1:33, 1:33],
                        x_bf[:, ct * HW:(ct + 1) * HW].rearrange(
                            "p (h w) -> p h w", h=H))

                # ---- K_pad / Q_pad ----
                k_pad = imgp.tile([128, 2 * HW], BF16, tag="kpad", name=f"kpad_{img}")
                q_pad = imgp.tile([128, 2 * HW], BF16, tag="qpad", name=f"qpad_{img}")
                for hh in range(2):
                    for tgt, dst in ((0, k_pad), (1, q_pad)):
                        for qn in range(2):
                            kqps = psp.tile([128, 512], F32, tag="ms",
                                            name=f"kqps_{img}_{tgt}_{hh}_{qn}")
                            for ct in range(2):
                                nc.tensor.matmul(
                                    kqps[:],
                                    wkq_pad[:, ct * 512 + tgt * 256 + hh * 128:
                                            ct * 512 + tgt * 256 + (hh + 1) * 128],
                                    x_bf[:, ct * HW + qn * 512:
                                         ct * HW + (qn + 1) * 512],
                                    start=(ct == 0), stop=False)
                            nc.tensor.matmul(
                                kqps[:],
                                brow_pad[0:1, (tgt * 2 + hh) * 128:
                                         (tgt * 2 + hh + 1) * 128],
                                ones512[0:1, :],
                                start=False, stop=True)
                            nc.vector.tensor_copy(
                                dst[:, hh * HW + qn * 512:
                                    hh * HW + (qn + 1) * 512], kqps[:])

                # ---- V^T with ones column, 32-stride padded blocks ----
                # vt_aug block (hh,kt) at cols (hh*8+kt)*128 + hp*32 +
                #   [0 = ones, 1:16 = zeros, 16:32 = V_h]  (M=32 AV matmuls
                #   write full PSUM quadrants; denominator lands on quadrant
                #   rows 32hp, attn on rows 32hp+16..32)
                vt_aug = imgp.tile([128, 2 * 8 * 128], BF16, tag="vtaug",
                                   name=f"vtaug_{img}")
                # pad value 1e-4 (not 0) keeps the reciprocal of pad rows
                # finite; proj weights for pad rows are exactly 0 so the
                # values never reach the output
                nc.gpsimd.memset(vt_aug[:], 1e-4)
                nc.gpsimd.memset(
                    vt_aug[:].rearrange("p (g d) -> p g d", d=32)[:, :, 0:1], 1.0)
                for kt in range(8):
                    vtps = psp.tile([128, 128], F32, tag="ms", name=f"vtps_{img}_{kt}")
                    for ct in range(2):
                        nc.tensor.matmul(
                            vtps[:],
                            x_bf[:, ct * HW + kt * 128: ct * HW + (kt + 1) * 128],
                            wvT[:, ct * 128:(ct + 1) * 128],
                            start=(ct == 0), stop=(ct == 1))
                    for hh in range(2):
                        base = (hh * 8 + kt) * 128
                        dst = vt_aug[:, base: base + 128].rearrange(
                            "p (h d) -> p h d", d=32)[:, :, 16:32]
                        src = vtps[:, hh * 64:(hh + 1) * 64].rearrange(
                            "p (h d) -> p h d", d=16)
                        bvb = bv_bc[:, hh * 64:(hh + 1) * 64].rearrange(
                            "p (h d) -> p h d", d=16)
                        nc.vector.tensor_add(dst, src, bvb)

                # ---- conv branch ----
                # computed over the padded flat space in row-aligned chunks so
                # every matmul rhs is a single contiguous run; junk columns
                # (x=32,33 of each padded row) are skipped on evacuation.
                out_conv = imgp.tile([128, HW], F32, tag="oconv", name=f"oconv_{img}")
                for (r0, nr) in ((0, 15), (15, 15), (30, 2)):
                    n = (nr - 1) * HP + W          # chunk free size (<=512)
                    cs = (r0 + 1) * HP + 1         # pad-flat offset of (r0, 0)
                    cvps = psp.tile([128, 512], F32, tag="ms",
                                    name=f"cvps_{img}_{r0}")
                    for t in range(9):
                        dy, dx = t // 3, t % 3
                        sh = (dy - 1) * HP + (dx - 1)
                        for ct in range(2):
                            nc.tensor.matmul(
                                cvps[:, 0:n],
                                wconv[:, (ct * 9 + t) * 128:(ct * 9 + t + 1) * 128],
                                xpad[:, ct * PADW + cs + sh: ct * PADW + cs + sh + n],
                                start=((t, ct) == (0, 0)), stop=False)
                    nc.tensor.matmul(cvps[:, 0:n], convb_row[:], ones512[0:1, 0:n],
                                     start=False, stop=True)
                    nc.vector.tensor_copy(
                        out_conv[:, r0 * W:(r0 + nr) * W].rearrange(
                            "p (h w) -> p h w", h=nr),
                        cvps[:, 0:nr * HP].rearrange(
                            "p (h w) -> p h w", w=HP)[:, :, 0:W])
                nc.sync.dma_start(out_ext[imgd, 0:CONV_CO, :], out_conv[:])

                # ---- attention ----
                # attn_pad: [128, 2*HW] bf16, hh at cols hh*HW; head hp's
                # normalized attn at rows 32hp+16..32 (fully written by the
                # normalize muls, so no memset needed)
                attn_pad = imgp.tile([128, 2 * HW], BF16, tag="attnp",
                                     name=f"attnp_{img}")
                rrec = imgp.tile([128, HW], F32, tag="rrec", name=f"rrec_{img}")
                rrec_bf = imgp.tile([128, HW], BF16, tag="rrecbf",
                                    name=f"rrecbf_{img}")
                for hh in range(2):
                    av = psp.tile([128, HW], F32, tag="av", bufs=1, name=f"av_{img}_{hh}")
                    for kt in range(8):
                        for qh in range(2):
                            lgs = []
                            for hg in range(2):
                                lg = psp.tile([128, 1024], F32, tag="lg", bufs=2,
                                              name=f"lg_{img}_{hh}_{kt}_{qh}_{hg}")
                                lgs.append(lg)
                                for j in range(2):
                                    hp = 2 * hg + j
                                    nc.tensor.matmul(
                                        lg[:, j * 512:(j + 1) * 512],
                                        k_pad[32 * hp:32 * hp + 16,
                                              hh * HW + kt * 128: hh * HW + (kt + 1) * 128],
                                        q_pad[32 * hp:32 * hp + 16,
                                              hh * HW + qh * 512: hh * HW + (qh + 1) * 512],
                                        start=True, stop=True,
                                        tile_position=(32 * hp, 0))
                            sts = []
                            for hg in range(2):
                                st = stp.tile([128, 1024], BF16, tag="st",
                                              name=f"st_{img}_{hh}_{kt}_{qh}_{hg}")
                                sts.append(st)
                                nc.scalar.activation(st[:], lgs[hg][:], AF.Exp,
                                                     scale=SCALE)
                            for hg in range(2):
                                for j in range(2):
                                    hp = 2 * hg + j
                                    base = (hh * 8 + kt) * 128 + 32 * hp
                                    nc.tensor.matmul(
                                        av[32 * hp:32 * hp + 32,
                                           qh * 512:(qh + 1) * 512],
                                        vt_aug[:, base: base + 32],
                                        sts[hg][:, j * 512:(j + 1) * 512],
                                        start=(kt == 0), stop=(kt == 7),
                                        skip_group_check=True,
                                        tile_position=(0, 32 * hp))
                    # evacuate av to SBUF right away so the PSUM slot frees,
                    # then normalize off the critical path.
                    # av rows per quadrant hp: 32hp = den, 32hp+1..16 = 0,
                    # 32hp+16..32 = unnormalized attn. Full-tile ops only.
                    av_sb = imgp.tile([128, HW], F32, tag="avsb",
                                      name=f"avsb_{img}_{hh}")
                    nc.vector.tensor_copy(av_sb[:], av[:])
                    for qh in range(2):
                        sl = slice(qh * 512, (qh + 1) * 512)
                        slh = slice(hh * HW + qh * 512, hh * HW + (qh + 1) * 512)
                        nc.vector.reciprocal(rrec[:, sl], av_sb[:, sl])
                        nc.vector.tensor_copy(rrec_bf[:, sl], rrec[:, sl])
                        # broadcast each quadrant's 1/den row via K=1 bf16
                        # matmuls on independent (row,col) PE tiles
                        rdps = psp.tile([128, 512], F32, tag="ms",
                                        name=f"rdps_{img}_{hh}_{qh}")
                        for hp in range(4):
                            nc.tensor.matmul(
                                rdps[32 * hp:32 * hp + 32, :],
                                ones_q[32 * hp:32 * hp + 1, :],
                                rrec_bf[32 * hp:32 * hp + 1, sl],
                                start=True, stop=True,
                                tile_position=(32 * hp, 32 * hp))
                        nc.vector.tensor_mul(attn_pad[:, slh], av_sb[:, sl],
                                             rdps[:])

                # ---- proj (padded weights over both halves) ----
                out_proj = imgp.tile([128, HW], F32, tag="oproj", name=f"oproj_{img}")
                for qn in range(2):
                    projps = psp.tile([128, 512], F32, tag="ms",
                                      name=f"projps_{img}_{qn}")
                    for hh in range(2):
                        nc.tensor.matmul(
                            projps[:],
                            projT_pad[:, hh * 128:(hh + 1) * 128],
                            attn_pad[:, hh * HW + qn * 512: hh * HW + (qn + 1) * 512],
                            start=(hh == 0), stop=False)
                    nc.tensor.matmul(projps[:], projb_row[:], ones512[0:1, :],
                                     start=False, stop=True)
                    nc.vector.tensor_copy(
                        out_proj[:, qn * 512:(qn + 1) * 512], projps[:])
                nc.sync.dma_start(out_ext[imgd, CONV_CO:, :], out_proj[:])

    return nc


_NC = None


def _get_nc():
    global _NC
    if _NC is None:
        _NC = build_nc()
        _NC.compile()
    return _NC


def kernel(**inputs):
    from concourse.bass_utils import run_bass_kernel_spmd

    nc = _get_nc()
    x = np.asarray(inputs["x"], np.float32).reshape(B, C, HW)
    conv_w = np.ascontiguousarray(np.asarray(inputs["conv_w"], np.float32).reshape(9, C, CONV_CO))
    conv_b = np.ascontiguousarray(np.asarray(inputs["conv_b"], np.float32).reshape(1, CONV_CO))
    qkv_w = np.ascontiguousarray(np.asarray(inputs["qkv_w"], np.float32))
    qkv_b = np.ascontiguousarray(np.asarray(inputs["qkv_b"], np.float32).reshape(1, 2 * DK + DV))
    proj_w = np.ascontiguousarray(np.asarray(inputs["proj_w"], np.float32))
    proj_b = np.ascontiguousarray(np.asarray(inputs["proj_b"], np.float32).reshape(1, DV))

    in_maps = []
    for i in range(N_CORES):
        in_maps.append({
            "x": np.ascontiguousarray(x[i * BL:(i + 1) * BL]),
            "conv_w": conv_w, "conv_b": conv_b,
            "qkv_w": qkv_w, "qkv_b": qkv_b,
            "proj_w": proj_w, "proj_b": proj_b,
        })
    res = run_bass_kernel_spmd(nc, in_maps, core_ids=list(range(N_CORES)))
    outs = [np.asarray(res.results[i]["out"]).reshape(BL, CO, H, W)
            for i in range(N_CORES)]
    return np.concatenate(outs, axis=0).astype(np.float32)


if __name__ == "__main__":
    nc = build_nc()
    nc.compile()
    print("built ok; instructions:", len(nc.inst_map))


# revision 55
# speedup vs baseline: 598.9236x; 9.7980x over previous
"""AAConv (attention-augmented conv) Trainium2 kernel, 8-core data-parallel.

Reference shapes: x (16,256,32,32) f32
  conv branch: 3x3 SAME conv 256->128 (+bias)
  attn branch: 1x1 qkv conv (k|q|v = 128|128|128 rows of qkv_w), 8 heads d=16,
               softmax attention over 1024 positions, 1x1 proj 128->128 (+bias)
  out = concat([conv_out, attn_out], axis=1) -> (16,256,32,32)

Sharding: pure data-parallel over batch. Each of 8 cores gets 2 images and
all weights; outputs concatenated on host.

Per-core design (channels on partitions, pixels on free dim):
 - logits computed transposed, L^T[k,q] (lhsT=K_h [16,128], rhs=Q_h [16,512]),
   4-way row-tiled over heads (K=16 contraction, tile_position=(32h,0)).
   K/Q stored padded: head h at partitions 32h..32h+16 (zero pad rows) so
   lhsT/rhs share base partitions; produced by M=128 matmuls against
   zero-padded transposed weights.
 - softmax denominator via a ones-column appended to V^T in the AV matmul
   (M=17 col-tiled, tile_position=(0,32h)): no cross-partition reductions.
 - exp on ScalarE (the bottleneck engine, ~128us/core) straight out of PSUM.
 - softmax normalize: strided-partition reciprocal on DVE + gpsimd
   partition_broadcast + one strided multiply; attn kept in the padded
   partition layout and proj done with zero-padded proj weights.
 - conv as 9 shift-matmuls over a zero-padded 34x34 spatial buffer.
 - all matmuls bf16 (f32 psum accumulate).
"""

import sys

for p in ("/opt/trn_rl_repo",):
    if p not in sys.path:
        sys.path.insert(0, p)

import numpy as np

import concourse.bass as bass
import concourse.tile as tile
from concourse import bacc, mybir
from concourse.masks import make_identity

F32 = mybir.dt.float32
BF16 = mybir.dt.bfloat16
AF = mybir.ActivationFunctionType

# Problem dims (hardcoded)
B, C, H, W = 16, 256, 32, 32
HW = H * W                      # 1024
CO, DK, DV, NH = 256, 128, 128, 8
D = DK // NH                    # 16 head dim
CONV_CO = CO - DV               # 128
N_CORES = 8
BL = B // N_CORES               # 2 images per core
HP = H + 2                      # 34 padded
PADHW = HP * HP                 # 1156
SCALE = float(D) ** -0.5        # 0.25


def hview(ap):
    """[128, N] -> [4, 32, N] partition-group view."""
    return ap.rearrange("(a b) n -> a b n", b=32)


def build_nc():
    nc = bacc.Bacc("TRN2", target_bir_lowering=False, debug=False,
                   num_devices=N_CORES)

    x_ext = nc.declare_dram_parameter("x", [BL, C, HW], F32, isOutput=False)
    convw_ext = nc.declare_dram_parameter("conv_w", [9, C, CONV_CO], F32, isOutput=False)
    convb_ext = nc.declare_dram_parameter("conv_b", [1, CONV_CO], F32, isOutput=False)
    qkvw_ext = nc.declare_dram_parameter("qkv_w", [2 * DK + DV, C], F32, isOutput=False)
    qkvb_ext = nc.declare_dram_parameter("qkv_b", [1, 2 * DK + DV], F32, isOutput=False)
    projw_ext = nc.declare_dram_parameter("proj_w", [DV, DV], F32, isOutput=False)
    projb_ext = nc.declare_dram_parameter("proj_b", [1, DV], F32, isOutput=False)
    out_ext = nc.declare_dram_parameter("out", [BL, CO, HW], F32, isOutput=True)

    with tile.TileContext(nc) as tc:
        with (
            tc.tile_pool(name="const", bufs=1) as constp,
            tc.tile_pool(name="stage", bufs=1) as stagep,
            tc.tile_pool(name="img", bufs=3) as imgp,
            tc.tile_pool(name="st", bufs=6) as stp,
            tc.tile_pool(name="psum", bufs=1, space="PSUM") as psp,
        ):
            # ---------------- weights ----------------
            ident = constp.tile([128, 128], F32)
            make_identity(nc, ident[:])

            # qkv weights: DMA natural [chan, c], PE-transpose to [c, chan],
            # scatter into zero-padded layouts.
            qkvw_sb = stagep.tile([128, 3 * C], F32)  # blk b at cols b*256
            for blk in range(3):
                nc.sync.dma_start(
                    qkvw_sb[:, blk * C:(blk + 1) * C],
                    qkvw_ext[blk * 128:(blk + 1) * 128, :],
                )

            # prefetch all images' x early (longest DMA pole) + compact bf16
            import os as _os
            _reps = int(_os.environ.get("AACONV_BENCH_REPS", "1"))
            xs_bf = []
            for img in range(BL * _reps):
                xin = imgp.tile([128, 2 * HW], F32, tag="xin", name=f"xin_{img}")
                for ct in range(2):
                    nc.sync.dma_start(
                        xin[:, ct * HW:(ct + 1) * HW],
                        x_ext[img % BL, ct * 128:(ct + 1) * 128, :])
                x_bf = imgp.tile([128, 2 * HW], BF16, tag="xbf", name=f"xbf_{img}")
                nc.vector.tensor_copy(x_bf[:], xin[:])
                xs_bf.append(x_bf)

            # wkq_pad: [128 c, ct*512 + tgt*256 + hh*128 + hp*32 + d] bf16, zero pad
            wkq_pad = constp.tile([128, 2 * 512], BF16)
            nc.gpsimd.memset(wkq_pad[:], 0.0)
            wvT = constp.tile([128, 2 * 128], BF16)
            for ct in range(2):
                for blk in range(3):
                    tps = psp.tile([128, 128], F32, tag="ms", name=f"tps_{ct}_{blk}")
                    nc.tensor.transpose(
                        tps[:], qkvw_sb[:, blk * C + ct * 128: blk * C + (ct + 1) * 128],
                        ident[:])
                    if blk < 2:
                        dst = wkq_pad[:, ct * 512 + blk * 256:
                                      ct * 512 + (blk + 1) * 256].rearrange(
                            "p (h d) -> p h d", d=32)[:, :, 0:16]
                        src = tps[:].rearrange("p (h d) -> p h d", d=16)
                        nc.vector.tensor_copy(dst, src)
                    else:
                        nc.vector.tensor_copy(
                            wvT[:, ct * 128:(ct + 1) * 128], tps[:])

            projw_sb = stagep.tile([128, 128], F32)
            nc.sync.dma_start(projw_sb[:], projw_ext[:])
            # padded projT: rows 32hp+16+d = proj_w^T row (4hh+hp)*16+d, rest 0
            # (matches the attn_pad layout where attn lives at rows 32hp+16..32).
            # Column-scatter proj_w in free space first, then PE-transpose.
            projw_pad = stagep.tile([128, 2 * 128], F32)
            nc.gpsimd.memset(projw_pad[:], 0.0)
            for hh in range(2):
                nc.vector.tensor_copy(
                    projw_pad[:, hh * 128:(hh + 1) * 128].rearrange(
                        "p (a b) -> p a b", b=32)[:, :, 16:32],
                    projw_sb[:, 64 * hh:64 * (hh + 1)].rearrange(
                        "p (a b) -> p a b", b=16))
            projT_pad = constp.tile([128, 2 * 128], BF16)
            for hh in range(2):
                tps2 = psp.tile([128, 128], F32, tag="ms", name=f"tps2_{hh}")
                nc.tensor.transpose(
                    tps2[:], projw_pad[:, hh * 128:(hh + 1) * 128], ident[:])
                nc.vector.tensor_copy(projT_pad[:, hh * 128:(hh + 1) * 128],
                                      tps2[:])

            # k/q biases as a padded bias ROW (free-dim scatter, all DVE-legal),
            # added to kqps via a K=1 matmul against a ones row.
            qkvb_sb = stagep.tile([1, 2 * DK + DV], F32)
            nc.sync.dma_start(qkvb_sb[:], qkvb_ext[:])
            brow_pad = constp.tile([1, 512], BF16)
            nc.gpsimd.memset(brow_pad[:], 0.0)
            for tgt in range(2):
                for hh in range(2):
                    nc.vector.tensor_copy(
                        brow_pad[0:1, (tgt * 2 + hh) * 128:
                                 (tgt * 2 + hh + 1) * 128].rearrange(
                            "p (a b) -> p a b", b=32)[:, :, 0:16],
                        qkvb_sb[0:1, tgt * DK + 64 * hh: tgt * DK + 64 * (hh + 1)
                                ].rearrange("p (a b) -> p a b", b=16))
            ones512 = constp.tile([1, 512], BF16)
            nc.gpsimd.memset(ones512[:], 1.0)
            convb_f32 = stagep.tile([1, CONV_CO], F32)
            nc.sync.dma_start(convb_f32[:], convb_ext[:])
            convb_row = constp.tile([1, CONV_CO], BF16)
            nc.vector.tensor_copy(convb_row[:], convb_f32[:])
            projb_f32 = stagep.tile([1, DV], F32)
            nc.sync.dma_start(projb_f32[:], projb_ext[:])
            projb_row = constp.tile([1, DV], BF16)
            nc.vector.tensor_copy(projb_row[:], projb_f32[:])

            # v-bias broadcast to 128 partitions via PE (ones ⊗ bv)
            bv_f32 = stagep.tile([1, DV], F32)
            nc.sync.dma_start(bv_f32[:], qkvb_ext[:, 2 * DK:])
            bv_bf = stagep.tile([1, DV], BF16)
            nc.vector.tensor_copy(bv_bf[:], bv_f32[:])
            ones_row = constp.tile([1, 128], BF16)
            nc.gpsimd.memset(ones_row[:], 1.0)
            ones_q = constp.tile([128, 32], BF16)
            nc.gpsimd.memset(ones_q[:], 1.0)
            bvps = psp.tile([128, 128], F32, tag="ms")
            nc.tensor.matmul(bvps[:], ones_row[:], bv_bf[:], start=True, stop=True)
            bv_bc = constp.tile([128, 128], F32)
            nc.vector.tensor_copy(bv_bc[:], bvps[:])

            # conv weights: natural [c, o] per tap, bf16. cols (ct*9+t)*128+o
            wconv_f32 = stagep.tile([128, 2 * 9 * CONV_CO], F32)
            wconv = constp.tile([128, 2 * 9 * CONV_CO], BF16)
            for ct in range(2):
                for t in range(9):
                    blk = slice((ct * 9 + t) * 128, (ct * 9 + t + 1) * 128)
                    nc.sync.dma_start(wconv_f32[:, blk],
                                      convw_ext[t, ct * 128:(ct + 1) * 128, :])
                    nc.vector.tensor_copy(wconv[:, blk], wconv_f32[:, blk])

            # ---------------- per image ----------------
            PADW = PADHW + 36   # room for the last conv chunk's shifted reads
            for img in range(BL * _reps):
                imgd = img % BL
                x_bf = xs_bf[img]
                # zero-padded 34x34 layout for the conv, filled via DMA
                xpad = imgp.tile([128, 2 * PADW], BF16, tag="xpad", name=f"xpad_{img}")
                nc.gpsimd.memset(xpad[:], 0.0)
                for ct in range(2):
                    nc.sync.dma_start(
                        xpad[:, ct * PADW: ct * PADW + PADHW].rearrange(
                            "p (h w) -> p h w", h=HP)[:, 1:33, 1:33],
                        x_bf[:, ct * HW:(ct + 1) * HW].rearrange(
                            "p (h w) -> p h w", h=H))

                # ---- K_pad / Q_pad ----
                k_pad = imgp.tile([128, 2 * HW], BF16, tag="kpad", name=f"kpad_{img}")
                q_pad = imgp.tile([128, 2 * HW], BF16, tag="qpad", name=f"qpad_{img}")
                for hh in range(2):
                    for tgt, dst in ((0, k_pad), (1, q_pad)):
                        for qn in range(2):
                            kqps = psp.tile([128, 512], F32, tag="ms",
                                            name=f"kqps_{img}_{tgt}_{hh}_{qn}")
                            for ct in range(2):
                                nc.tensor.matmul(
                                    kqps[:],
                                    wkq_pad[:, ct * 512 + tgt * 256 + hh * 128:
                                            ct * 512 + tgt * 256 + (hh + 1) * 128],
                                    x_bf[:, ct * HW + qn * 512:
                                         ct * HW + (qn + 1) * 512],
                                    start=(ct == 0), stop=False)
                            nc.tensor.matmul(
                                kqps[:],
                                brow_pad[0:1, (tgt * 2 + hh) * 128:
                                         (tgt * 2 + hh + 1) * 128],
                                ones512[0:1, :],
                                start=False, stop=True)
                            nc.vector.tensor_copy(
                                dst[:, hh * HW + qn * 512:
                                    hh * HW + (qn + 1) * 512], kqps[:])

                # ---- V^T with ones column, 32-stride padded blocks ----
                # vt_aug block (hh,kt) at cols (hh*8+kt)*128 + hp*32 +
                #   [0 = ones, 1:16 = zeros, 16:32 = V_h]  (M=32 AV matmuls
                #   write full PSUM quadrants; denominator lands on quadrant
                #   rows 32hp, attn on rows 32hp+16..32)
                vt_aug = imgp.tile([128, 2 * 8 * 128], BF16, tag="vtaug",
                                   name=f"vtaug_{img}")
                # pad value 1e-4 (not 0) keeps the reciprocal of pad rows
                # finite; proj weights for pad rows are exactly 0 so the
                # values never reach the output
                nc.gpsimd.memset(vt_aug[:], 1e-4)
                nc.gpsimd.memset(
                    vt_aug[:].rearrange("p (g d) -> p g d", d=32)[:, :, 0:1], 1.0)
                for kt in range(8):
                    vtps = psp.tile([128, 128], F32, tag="ms", name=f"vtps_{img}_{kt}")
                    for ct in range(2):
                        nc.tensor.matmul(
                            vtps[:],
                            x_bf[:, ct * HW + kt * 128: ct * HW + (kt + 1) * 128],
                            wvT[:, ct * 128:(ct + 1) * 128],
                            start=(ct == 0), stop=(ct == 1))
                    for hh in range(2):
                        base = (hh * 8 + kt) * 128
                        dst = vt_aug[:, base: base + 128].rearrange(
                            "p (h d) -> p h d", d=32)[:, :, 16:32]
                        src = vtps[:, hh * 64:(hh + 1) * 64].rearrange(
                            "p (h d) -> p h d", d=16)
                        bvb = bv_bc[:, hh * 64:(hh + 1) * 64].rearrange(
                            "p (h d) -> p h d", d=16)
                        nc.vector.tensor_add(dst, src, bvb)

                # ---- conv branch ----
                # computed over the padded flat space in row-aligned chunks so
                # every matmul rhs is a single contiguous run; junk columns
                # (x=32,33 of each padded row) are skipped on evacuation.
                out_conv = imgp.tile([128, HW], F32, tag="oconv", name=f"oconv_{img}")
                for (r0, nr) in ((0, 15), (15, 15), (30, 2)):
                    n = (nr - 1) * HP + W          # chunk free size (<=512)
                    cs = (r0 + 1) * HP + 1         # pad-flat offset of (r0, 0)
                    cvps = psp.tile([128, 512], F32, tag="ms",
                                    name=f"cvps_{img}_{r0}")
                    for t in range(9):
                        dy, dx = t // 3, t % 3
                        sh = (dy - 1) * HP + (dx - 1)
                        for ct in range(2):
                            nc.tensor.matmul(
                                cvps[:, 0:n],
                                wconv[:, (ct * 9 + t) * 128:(ct * 9 + t + 1) * 128],
                                xpad[:, ct * PADW + cs + sh: ct * PADW + cs + sh + n],
                                start=((t, ct) == (0, 0)), stop=False)
                    nc.tensor.matmul(cvps[:, 0:n], convb_row[:], ones512[0:1, 0:n],
                                     start=False, stop=True)
                    nc.vector.tensor_copy(
                        out_conv[:, r0 * W:(r0 + nr) * W].rearrange(
                            "p (h w) -> p h w", h=nr),
                        cvps[:, 0:nr * HP].rearrange(
                            "p (h w) -> p h w", w=HP)[:, :, 0:W])
                nc.sync.dma_start(out_ext[imgd, 0:CONV_CO, :], out_conv[:])

                # ---- attention ----
                # attn_pad: [128, 2*HW] bf16, hh at cols hh*HW; head hp's
                # normalized attn at rows 32hp+16..32 (fully written by the
                # normalize muls, so no memset needed)
                attn_pad = imgp.tile([128, 2 * HW], BF16, tag="attnp",
                                     name=f"attnp_{img}")
                rrec = imgp.tile([128, HW], F32, tag="rrec", name=f"rrec_{img}")
                rrec_bf = imgp.tile([128, HW], BF16, tag="rrecbf",
                                    name=f"rrecbf_{img}")
                # qh-serial: av is one PSUM bank, which frees banks for
                # lg bufs=3 (a full lookahead tile -> no ACT ping-pong stall)
                for hh in range(2):
                    for qh in range(2):
                        sl = slice(qh * 512, (qh + 1) * 512)
                        slh = slice(hh * HW + qh * 512, hh * HW + (qh + 1) * 512)
                        av = psp.tile([128, 512], F32, tag="av", bufs=1,
                                      name=f"av_{img}_{hh}_{qh}")
                        for kt in range(8):
                            lgs = []
                            for hg in range(2):
                                lg = psp.tile([128, 1024], F32, tag="lg", bufs=3,
                                              name=f"lg_{img}_{hh}_{qh}_{kt}_{hg}")
                                lgs.append(lg)
                                for j in range(2):
                                    hp = 2 * hg + j
                                    nc.tensor.matmul(
                                        lg[:, j * 512:(j + 1) * 512],
                                        k_pad[32 * hp:32 * hp + 16,
                                              hh * HW + kt * 128: hh * HW + (kt + 1) * 128],
                                        q_pad[32 * hp:32 * hp + 16,
                                              hh * HW + qh * 512: hh * HW + (qh + 1) * 512],
                                        start=True, stop=True,
                                        tile_position=(32 * hp, 0))
                            sts = []
                            for hg in range(2):
                                st = stp.tile([128, 1024], BF16, tag="st",
                                              name=f"st_{img}_{hh}_{qh}_{kt}_{hg}")
                                sts.append(st)
                                nc.scalar.activation(st[:], lgs[hg][:], AF.Exp,
                                                     scale=SCALE)
                            for hg in range(2):
                                for j in range(2):
                                    hp = 2 * hg + j
                                    base = (hh * 8 + kt) * 128 + 32 * hp
                                    nc.tensor.matmul(
                                        av[32 * hp:32 * hp + 32, :],
                                        vt_aug[:, base: base + 32],
                                        sts[hg][:, j * 512:(j + 1) * 512],
                                        start=(kt == 0), stop=(kt == 7),
                                        skip_group_check=True,
                                        tile_position=(0, 32 * hp))
                        # evacuate av to SBUF right away so the PSUM bank
                        # frees, then normalize off the critical path.
                        # av rows per quadrant hp: 32hp = den, +1..16 = pad,
                        # +16..32 = unnormalized attn. Full-tile ops only.
                        av_sb = imgp.tile([128, 512], F32, tag="avsb",
                                          name=f"avsb_{img}_{hh}_{qh}")
                        nc.vector.tensor_copy(av_sb[:], av[:])
                        nc.vector.reciprocal(rrec[:, sl], av_sb[:])
                        nc.vector.tensor_copy(rrec_bf[:, sl], rrec[:, sl])
                        # broadcast each quadrant's 1/den row via K=1 bf16
                        # matmuls on independent (row,col) PE tiles
                        rdps = psp.tile([128, 512], F32, tag="ms",
                                        name=f"rdps_{img}_{hh}_{qh}")
                        for hp in range(4):
                            nc.tensor.matmul(
                                rdps[32 * hp:32 * hp + 32, :],
                                ones_q[32 * hp:32 * hp + 1, :],
                                rrec_bf[32 * hp:32 * hp + 1, sl],
                                start=True, stop=True,
                                tile_position=(32 * hp, 32 * hp))
                        nc.vector.tensor_mul(attn_pad[:, slh], av_sb[:],
                                             rdps[:])

                # ---- proj (padded weights over both halves) ----
                out_proj = imgp.tile([128, HW], F32, tag="oproj", name=f"oproj_{img}")
                for qn in range(2):
                    projps = psp.tile([128, 512], F32, tag="ms",
                                      name=f"projps_{img}_{qn}")
                    for hh in range(2):
                        nc.tensor.matmul(
                            projps[:],
                            projT_pad[:, hh * 128:(hh + 1) * 128],
                            attn_pad[:, hh * HW + qn * 512: hh * HW + (qn + 1) * 512],
                            start=(hh == 0), stop=False)
                    nc.tensor.matmul(projps[:], projb_row[:], ones512[0:1, :],
                                     start=False, stop=True)
                    nc.vector.tensor_copy(
                        out_proj[:, qn * 512:(qn + 1) * 512], projps[:])
                nc.sync.dma_start(out_ext[imgd, CONV_CO:, :], out_proj[:])

    return nc


_NC = None


def _get_nc():
    global _NC
    if _NC is None:
        _NC = build_nc()
        _NC.compile()
    return _NC


def kernel(**inputs):
    from concourse.bass_utils import run_bass_kernel_spmd

    nc = _get_nc()
    x = np.asarray(inputs["x"], np.float32).reshape(B, C, HW)
    conv_w = np.ascontiguousarray(np.asarray(inputs["conv_w"], np.float32).reshape(9, C, CONV_CO))
    conv_b = np.ascontiguousarray(np.asarray(inputs["conv_b"], np.float32).reshape(1, CONV_CO))
    qkv_w = np.ascontiguousarray(np.asarray(inputs["qkv_w"], np.float32))
    qkv_b = np.ascontiguousarray(np.asarray(inputs["qkv_b"], np.float32).reshape(1, 2 * DK + DV))
    proj_w = np.ascontiguousarray(np.asarray(inputs["proj_w"], np.float32))
    proj_b = np.ascontiguousarray(np.asarray(inputs["proj_b"], np.float32).reshape(1, DV))

    in_maps = []
    for i in range(N_CORES):
        in_maps.append({
            "x": np.ascontiguousarray(x[i * BL:(i + 1) * BL]),
            "conv_w": conv_w, "conv_b": conv_b,
            "qkv_w": qkv_w, "qkv_b": qkv_b,
            "proj_w": proj_w, "proj_b": proj_b,
        })
    res = run_bass_kernel_spmd(nc, in_maps, core_ids=list(range(N_CORES)))
    outs = [np.asarray(res.results[i]["out"]).reshape(BL, CO, H, W)
            for i in range(N_CORES)]
    return np.concatenate(outs, axis=0).astype(np.float32)


if __name__ == "__main__":
    nc = build_nc()
    nc.compile()
    print("built ok; instructions:", len(nc.inst_map))
